# revision 24
# baseline (speedup 1.0000x reference)
"""Multi-head causal attention (B=2, S=2048, D=1024, H=16) on 8 NeuronCores.

Sharding: core c = (batch b=c//4, head-group g=c%4 of 4 heads).
All matmul operands in bf16 (host pre-casts x/w_qkv/w_out), fp32 PSUM
accumulation. Per core: project Q/K (transposed layout) and V for its 4
heads, run causal attention over all 2048 queries in transposed-score
layout ST[k, q], softmax denominator fused into the A@V matmul via a
ones-column in V. Two AllToAlls (one per head pair) swap head-shards for
query-shards; each core then runs the output projection on a fixed local
256-query slice of each batch.

Schedule: projections are fused chunk-major into the attention pipeline
(attention chunk j starts as soon as its Q/K/V chunks are projected);
score matmuls are emitted one group ahead of the exp->AV chain to keep
the PE dense; a tiny dummy AllToAll early in the program absorbs the
entry barrier and first-collective trigger warmup; A2A#1 fires right
after pair-0 attention; the output projection (even f-blocks first, odds
gated on A2A#2) is pinned after attention-B in every engine stream so a
late collective cannot head-of-line-block the PE.
"""

import numpy as np
import ml_dtypes

import concourse.bass as bass
import concourse.mybir as mybir
import concourse.tile as tile
from concourse import bacc
from concourse.bass_utils import run_bass_kernel_spmd

B, S, D = 2, 2048, 1024
H = 16
DH = 64  # head dim
N_CORES = 8
GROUPS = 4  # cores per batch = head groups
H_LOC = H // GROUPS  # 4 heads per core
EH = H_LOC * DH  # 256 local qkv width
QCH = 512  # query chunk
NCH = S // QCH  # 4
KB = 128  # key block
NKB = S // KB  # 16
NDB = D // 128  # 8 contraction blocks
QL = 256  # local output query rows per batch
VW = DH + 1  # 65: V columns + fused ones column
SCALE = 1.0 / 8.0  # 1/sqrt(DH)

F32 = mybir.dt.float32
BF16 = mybir.dt.bfloat16
MM_DT = BF16
EXP = mybir.ActivationFunctionType.Exp
MULT = mybir.AluOpType.mult
ADD = mybir.AluOpType.add


def _emit(nc, tc, xT, wq_d, wk_d, wv_d, wo_d, bb_d, y_d):
    from contextlib import ExitStack

    ctx = ExitStack()
    with ctx:
        persist = ctx.enter_context(tc.tile_pool(name="persist", bufs=1))
        psum_cm = tc.tile_pool(name="psum", bufs=1, space="PSUM")
        psum = psum_cm.__enter__()
        dram = ctx.enter_context(tc.tile_pool(name="dram", bufs=1, space="DRAM"))

        # --- weights first (gpsimd queue), x chunks (sync queue) ---
        w_sb = {
            nm: persist.tile([128, NDB * EH], MM_DT, name=f"w{nm}sb")
            for nm in ("q", "k", "v")
        }
        for nm, wd in (("q", wq_d), ("k", wk_d), ("v", wv_d)):
            nc.gpsimd.dma_start(
                w_sb[nm][:].rearrange("p (db e) -> p db e", db=NDB),
                wd.rearrange("(db p) e -> p db e", p=128),
            )
        xt = persist.tile([128, NDB * S], MM_DT, name="xt")
        xt_v = xt[:].rearrange("p (db s) -> p db s", db=NDB)
        xT_v = xT.rearrange("(db p) s -> p db s", p=128)
        for j in range(NCH):
            nc.sync.dma_start(
                xt_v[:, :, j * QCH : (j + 1) * QCH],
                xT_v[:, :, j * QCH : (j + 1) * QCH],
            )

        # --- constants: causal 0/1 masks ---
        # maskA: diag sub-blocks m=0,1 ; maskB: m=2,3. mask[ki, m*512+qi]=(qi>=ki+128m)
        maskA = persist.tile([128, 2 * QCH], MM_DT)
        maskB = persist.tile([128, 2 * QCH], MM_DT)
        for mt, m0 in ((maskA, 0), (maskB, 2)):
            nc.gpsimd.memset(mt[:], 1.0)
            for sub in range(2):
                m = m0 + sub
                nc.gpsimd.affine_select(
                    out=mt[:, sub * QCH : (sub + 1) * QCH],
                    in_=mt[:, sub * QCH : (sub + 1) * QCH],
                    compare_op=mybir.AluOpType.is_ge,
                    fill=0.0,
                    base=-128 * m,
                    channel_multiplier=-1,
                    pattern=[[1, QCH]],
                )
        ones_b = persist.tile([128, 8], MM_DT)
        nc.gpsimd.memset(ones_b[:], 1.0)

        # V with fused ones column: slice (kb, h) at (kb*H_LOC + h) * VW
        vgall = persist.tile([128, NKB * H_LOC * VW], MM_DT, name="vgall")
        nc.vector.tensor_copy(
            vgall[:].rearrange("p (n w) -> p n w", w=VW)[:, :, DH : DH + 1],
            ones_b[:, 0:1].unsqueeze(2).broadcast_to([128, NKB * H_LOC, 1]),
        )

        def vg(h, kb):
            i = (kb * H_LOC + h) * VW
            return vgall[:, i : i + VW]

        wo_sb = persist.tile([128, NDB * D], MM_DT)
        bb_sb = persist.tile([128, D], F32)

        # attention outputs for own heads, transposed: 2 tiles x [128 (2 heads), S]
        oft_own = [persist.tile([128, S], MM_DT, name=f"oftown{p}") for p in range(2)]
        qt = [persist.tile([128, S], MM_DT, name=f"qt{p}") for p in range(2)]
        kt = [persist.tile([128, S], MM_DT, name=f"kt{p}") for p in range(2)]

        exps = ctx.enter_context(tc.tile_pool(name="exps", bufs=1))

        def normalize(h, pot, j):
            p, r = h // 2, DH * (h % 2)
            den = exps.tile([1, QCH], F32, tag="den", bufs=2, name=f"den{h}_{j}")
            nc.vector.tensor_copy(den[:], pot[DH : DH + 1, :])
            rec = exps.tile([1, QCH], F32, tag="rec", bufs=2, name=f"rec{h}_{j}")
            nc.vector.reciprocal_approx_fast(rec[:], den[:])
            pb_sb = exps.tile([DH, QCH], F32, tag="pbsb", bufs=2, name=f"pb{h}_{j}")
            nc.gpsimd.partition_broadcast(pb_sb[:], rec[0:1, :])
            nc.vector.tensor_tensor(
                oft_own[p][r : r + DH, j * QCH : (j + 1) * QCH],
                pot[0:DH, :],
                pb_sb[:],
                op=MULT,
            )

        def emit_attention_pair(p, fillers=None, mid=None):
            h0, h1 = 2 * p, 2 * p + 1

            def emit_scores(j, g):
                pss = [
                    psum.tile(
                        [128, 2 * QCH], F32, tag="ps", bufs=3, name=f"ps{h}{j}{g}"
                    )
                    for h in (h0, h1)
                ]
                for sub in range(2):
                    kb = 2 * g + sub
                    m = kb - 4 * j
                    off = 128 * m if (g >= 2 * j and m > 0) else 0
                    for hi in range(2):
                        nc.tensor.matmul(
                            pss[hi][:, sub * QCH + off : (sub + 1) * QCH],
                            kt[p][hi * DH : (hi + 1) * DH, kb * KB : (kb + 1) * KB],
                            qt[p][
                                hi * DH : (hi + 1) * DH,
                                j * QCH + off : (j + 1) * QCH,
                            ],
                            start=True,
                            stop=True,
                        )
                return pss

            for j in range(NCH):
                nkb_j = 4 * (j + 1)
                pot = [
                    psum.tile([VW, QCH], F32, tag="pot", bufs=2, name=f"pot{h}_{j}")
                    for h in (h0, h1)
                ]
                gs = 2 * (j + 1)
                fill = list(fillers.get(j, ())) if fillers else []
                pss_next = emit_scores(j, 0)
                for g in range(gs):
                    pss = pss_next
                    pss_next = emit_scores(j, g + 1) if g + 1 < gs else None
                    if fill:
                        fill.pop(0)()
                    for hi, h in enumerate((h0, h1)):
                        e = exps.tile(
                            [128, 2 * QCH], MM_DT, tag="exp", bufs=3, name=f"e{h}{j}{g}"
                        )
                        # diagonal groups: leading 128*m0 columns of sub0 are
                        # fully masked; skip them in the exp (the mask multiply
                        # zeroes them from whatever stale data remains)
                        eoff = 128 * (2 * g - 4 * j) if g >= 2 * j else 0
                        nc.scalar.activation(
                            e[:, eoff:], pss[hi][:, eoff:], EXP, scale=SCALE
                        )
                        if g >= 2 * j:
                            mt = maskA if g == 2 * j else maskB
                            nc.vector.tensor_tensor(e[:], e[:], mt[:], op=MULT)
                        for sub in range(2):
                            kb = 2 * g + sub
                            m = kb - 4 * j
                            off = 128 * m if (g >= 2 * j and m > 0) else 0
                            nc.tensor.matmul(
                                pot[hi][:, off:QCH],
                                vg(h, kb),
                                e[:, sub * QCH + off : (sub + 1) * QCH],
                                start=(kb == 0),
                                stop=(kb == nkb_j - 1),
                            )
                for f in fill:  # leftover fillers
                    f()
                normalize(h0, pot[0], j)
                normalize(h1, pot[1], j)
                if mid is not None and j in mid:
                    mid[j]()

        def emit_proj_qk_unit(p, dst_i, j):
            dst, wsb = ((qt[p], w_sb["q"]), (kt[p], w_sb["k"]))[dst_i]
            ps = psum.tile([128, QCH], F32, tag="ps", bufs=3, name=f"pp{p}{dst_i}{j}")
            for d in range(NDB):
                nc.tensor.matmul(
                    ps[:],
                    wsb[:, d * EH + 128 * p : d * EH + 128 * p + 128],
                    xt[:, d * S + j * QCH : d * S + (j + 1) * QCH],
                    start=(d == 0),
                    stop=(d == NDB - 1),
                )
            nc.vector.tensor_copy(dst[:, j * QCH : (j + 1) * QCH], ps[:])

        def emit_proj_v_range(lo, hi):
            for sb_i in range(lo, hi):
                ps = psum.tile([128, EH], F32, tag="ps", bufs=3, name=f"pv{sb_i}")
                for d in range(NDB):
                    nc.tensor.matmul(
                        ps[:],
                        xt[:, d * S + sb_i * KB : d * S + (sb_i + 1) * KB],
                        w_sb["v"][:, d * EH : (d + 1) * EH],
                        start=(d == 0),
                        stop=(d == NDB - 1),
                    )
                dst = vgall[:, sb_i * H_LOC * VW : (sb_i + 1) * H_LOC * VW]
                nc.vector.tensor_copy(
                    dst.rearrange("p (h w) -> p h w", w=VW)[:, :, 0:DH],
                    ps[:].rearrange("p (h d) -> p h d", d=DH),
                )

        # --- AllToAll: shard s of pair-p buffer = this core's 2 heads' columns
        # q in [256s, 256s+256). Received shard from rank r lands at rows
        # [128r, 128r+128) = global f-rows [256r + 128p, +128) = f-block 2r+p.
        oft_all = [
            persist.tile([128, 2 * QL], MM_DT, name=f"oft{f}") for f in range(NDB)
        ]
        a2a_bufs = {}

        def emit_a2a_pre(p):
            cin = dram.tile([N_CORES * 128, QL], MM_DT, name=f"cin{p}")
            cout = dram.tile([N_CORES * 128, QL], MM_DT, name=f"cout{p}")
            a2a_bufs[p] = (cin, cout)
            nc.sync.dma_start(
                cin.rearrange("(s p) q -> p s q", p=128),
                oft_own[p][:].rearrange("p (s q) -> p s q", q=QL),
            )

        def emit_a2a_trigger(p):
            cin, cout = a2a_bufs[p]
            nc.gpsimd.collective_compute(
                "AllToAll",
                mybir.AluOpType.bypass,
                replica_groups=[list(range(N_CORES))],
                ins=[cin[:]],
                outs=[cout[:]],
            )

        def emit_a2a_post(p):
            cin, cout = a2a_bufs[p]
            cout_v = cout.rearrange("(b r p) q -> p b r q", p=128, b=2)
            for r in range(GROUPS):
                nc.sync.dma_start(
                    oft_all[2 * r + p][:].rearrange("p (b q) -> p b q", q=QL),
                    cout_v[:, :, r, :],
                )

        def qk(p, d, j):
            return lambda: emit_proj_qk_unit(p, d, j)

        def vj(sb):
            return lambda: emit_proj_v_range(sb, sb + 1)

        def pair(a, b):
            return lambda: (a(), b())

        def wo_bb():
            nc.gpsimd.dma_start(
                wo_sb[:].rearrange("p (db e) -> p db e", db=NDB),
                wo_d.rearrange("(db p) e -> p db e", p=128),
            )
            nc.sync.dma_start(bb_sb[:], bb_d[:])

        # just-in-time projection fillers: one per attention g-iteration, each
        # placed at least one g before its first consumer
        fillers_a = {
            0: [qk(0, 0, 1), qk(0, 1, 1)],
            1: [pair(vj(4), qk(0, 0, 2)), pair(vj(5), qk(0, 1, 2)), vj(6), vj(7)],
            2: [vj(8), vj(9), vj(10), vj(11), qk(0, 0, 3), qk(0, 1, 3)],
            3: [
                vj(12),
                vj(13),
                vj(14),
                vj(15),
                qk(1, 0, 0),
                qk(1, 1, 0),
                pair(qk(1, 0, 1), wo_bb),
                qk(1, 1, 1),
            ],
        }
        fillers_b = {
            0: [qk(1, 0, 2), qk(1, 1, 2)],
            1: [qk(1, 0, 3), qk(1, 1, 3)],
        }

        # --- schedule ---
        emit_proj_qk_unit(0, 0, 0)
        emit_proj_qk_unit(0, 1, 0)
        emit_proj_v_range(0, 4)
        emit_attention_pair(0, fillers=fillers_a)
        emit_a2a_pre(0)
        emit_a2a_trigger(0)
        emit_a2a_post(0)
        emit_attention_pair(1, fillers=fillers_b)
        emit_a2a_pre(1)
        emit_a2a_trigger(1)
        emit_a2a_post(1)
        psum_cm.__exit__(None, None, None)  # free attention PSUM for psum2
        psum2 = ctx.enter_context(tc.tile_pool(name="psum2", bufs=1, space="PSUM"))

        # --- output projection on local 256-query slice of each batch ---
        # pinned after attention in every engine stream (a late collective must
        # not head-of-line-block the PE). Two passes over all 8 PSUM groups:
        # even f-blocks (available after A2A#1) accumulate across every group
        # first, so the only work gated on A2A#2 is the odd-block pass.
        with tc.tile_wait_until(1.0):
            groups = [(bi, qb, ech) for bi in range(2) for qb in range(2)
                      for ech in range(2)]
            pys = {}
            for bi, qb, ech in groups:
                py = psum2.tile(
                    [128, 512], F32, tag="py", bufs=8, name=f"py{bi}{qb}{ech}"
                )
                pys[bi, qb, ech] = py
                for fi, f in enumerate([0, 2, 4, 6]):
                    nc.tensor.matmul(
                        py[:],
                        oft_all[f][
                            :, bi * QL + qb * 128 : bi * QL + (qb + 1) * 128
                        ],
                        wo_sb[:, f * D + ech * 512 : f * D + ech * 512 + 512],
                        start=(fi == 0),
                        stop=False,
                    )
            for bi in range(2):
                for qb in range(2):
                    ysb = persist.tile(
                        [128, D], F32, tag="ysb", bufs=2, name=f"y{bi}{qb}"
                    )
                    for ech in range(2):
                        py = pys[bi, qb, ech]
                        for fi, f in enumerate([1, 3, 5, 7]):
                            nc.tensor.matmul(
                                py[:],
                                oft_all[f][
                                    :, bi * QL + qb * 128 : bi * QL + (qb + 1) * 128
                                ],
                                wo_sb[:, f * D + ech * 512 : f * D + ech * 512 + 512],
                                start=False,
                                stop=(fi == 3),
                            )
                        nc.vector.tensor_tensor(
                            ysb[:, ech * 512 : (ech + 1) * 512],
                            py[:],
                            bb_sb[:, ech * 512 : (ech + 1) * 512],
                            op=ADD,
                        )
                    nc.sync.dma_start(
                        y_d[bi * QL + qb * 128 : bi * QL + (qb + 1) * 128, :], ysb[:]
                    )


def build_program():
    nc = bacc.Bacc(
        "TRN2", target_bir_lowering=False, debug=False, num_devices=N_CORES
    )
    xT = nc.dram_tensor("xT", [D, S], BF16, kind="ExternalInput")
    wq = nc.dram_tensor("wq", [D, EH], BF16, kind="ExternalInput")
    wk = nc.dram_tensor("wk", [D, EH], BF16, kind="ExternalInput")
    wv = nc.dram_tensor("wv", [D, EH], BF16, kind="ExternalInput")
    wo = nc.dram_tensor("wo", [D, D], BF16, kind="ExternalInput")
    bb = nc.dram_tensor("bb", [128, D], F32, kind="ExternalInput")
    y = nc.dram_tensor("y", [2 * QL, D], F32, kind="ExternalOutput")
    with tile.TileContext(nc) as tc:
        _emit(nc, tc, xT.ap(), wq.ap(), wk.ap(), wv.ap(), wo.ap(), bb.ap(), y.ap())
    nc.compile()
    return nc


_cached_nc = None


def _get_nc():
    global _cached_nc
    if _cached_nc is None:
        _cached_nc = build_program()
    return _cached_nc


def make_in_maps(x, w_qkv, w_out, b_out):
    bf = ml_dtypes.bfloat16
    x = np.asarray(x, np.float32)
    w_qkv = np.asarray(w_qkv, np.float32).astype(bf)
    w_out = np.ascontiguousarray(np.asarray(w_out, np.float32).astype(bf))
    b_out = np.asarray(b_out, np.float32)
    bb = np.ascontiguousarray(np.broadcast_to(b_out, (128, D)))
    xTb = [np.ascontiguousarray(x[b].T.astype(bf)) for b in range(B)]
    in_maps = []
    for c in range(N_CORES):
        b, g = c // GROUPS, c % GROUPS
        in_maps.append(
            {
                "xT": xTb[b],
                "wq": np.ascontiguousarray(w_qkv[:, g * EH : (g + 1) * EH]),
                "wk": np.ascontiguousarray(w_qkv[:, D + g * EH : D + (g + 1) * EH]),
                "wv": np.ascontiguousarray(
                    w_qkv[:, 2 * D + g * EH : 2 * D + (g + 1) * EH]
                ),
                "wo": w_out,
                "bb": bb,
            }
        )
    return in_maps


def assemble(results):
    # core c's y is [512, D]: rows [0,256) = batch 0 q-slice [256c, 256c+256),
    # rows [256,512) = batch 1 same slice.
    y = np.empty((B, S, D), np.float32)
    for c in range(N_CORES):
        yc = results[c]["y"]
        y[0, 256 * c : 256 * (c + 1), :] = yc[:256]
        y[1, 256 * c : 256 * (c + 1), :] = yc[256:]
    return y


def kernel(x, w_qkv, w_out, b_out, _trace=False, **run_kwargs):
    nc = _get_nc()
    in_maps = make_in_maps(x, w_qkv, w_out, b_out)
    res = run_bass_kernel_spmd(
        nc, in_maps, core_ids=list(range(N_CORES)), trace=_trace, **run_kwargs
    )
    out = assemble(res.results)
    if _trace:
        return out, res
    return out
